# revision 1
# baseline (speedup 1.0000x reference)
"""Contrastive (SimCLR-style) loss on 8 Trainium2 NeuronCores.

Math (matches the reference exactly):
  P = concat(projection1, projection2)            # [8192, 256]
  sim = cos_sim(P_i, P_j); diag masked to -1e9; logits = sim / 0.5
  labels = arange(2B)  -> picks the masked diagonal, so
  loss = -mean_i( logp_ii ),  logp_ii = f32(-2e9 - lse_i),
  lse_i = log(sum_{j != i} exp(2*sim_ij))

Distribution: data-parallel over the 8192 rows.  Each core receives the
full projection matrix (row-major fp32 for norms + pre-transposed bf16
for the matmul operand) plus its own 1024-row block.  On chip it:
  - computes row norms (DVE square+reduce, Newton rsqrt -- no ScalarE),
  - scales the transposed operand by 1/norm (bf16),
  - matmuls its row block against all 8192 columns (bf16, fp32 PSUM),
  - streams exp through ScalarE with fused row-sum accumulation,
  - subtracts the diagonal term and takes log.
Host all-reduces the per-row lse partials and applies the reference's
fp32 arithmetic for the final mean.
"""

import sys

for _p in ("/opt/trn_rl_repo", "/root/.axon_site/_ro/trn_rl_repo"):
    if _p not in sys.path:
        sys.path.append(_p)

import numpy as np

import concourse.bacc as bacc
import concourse.tile as tile
from concourse import mybir
from concourse import bass_utils

F32 = mybir.dt.float32
BF16 = mybir.dt.bfloat16
I32 = mybir.dt.int32
AF = mybir.ActivationFunctionType
ALU = mybir.AluOpType

N_CORES = 8
B = 8192          # total rows (2 * batch)
D = 256           # projection dim
BLK = B // N_CORES        # 1024 rows per core
M_TILES = BLK // 128      # 8 row tiles per core
N_COLS = 512              # matmul free dim (one PSUM bank)
GROUP = 2048              # ACT exp batch (4 PSUM banks) = one column group
N_GROUPS = B // GROUP     # 4
N_PER_GROUP = GROUP // N_COLS  # 4
U = 16                    # consecutive rows per partition in stats loads
RSQRT_MAGIC = 0x5F3759DF


def _newton_rsqrt(nc, pool, out_rn, s):
    """out_rn = 1/sqrt(s), entirely on VectorE (fp32).

    Quake-style bit seed + 2 Newton iterations (~5e-6 rel err).  Keeps
    ScalarE free for exp and avoids sqrt<->exp table reloads.
    """
    p, w = s.shape
    ibits = pool.tile([p, w], I32, name="ibits", tag="rsq_i", bufs=2)
    nc.vector.tensor_scalar(
        out=ibits, in0=s.bitcast(I32), scalar1=1, scalar2=None,
        op0=ALU.arith_shift_right,
    )
    nc.vector.tensor_scalar(
        out=ibits, in0=ibits, scalar1=-1, scalar2=RSQRT_MAGIC,
        op0=ALU.mult, op1=ALU.add,
    )
    y = ibits.bitcast(F32)
    t1 = pool.tile([p, w], F32, name="t1", tag="rsq_t1", bufs=2)
    for _ in range(2):
        nc.vector.tensor_mul(t1, y, y)
        nc.vector.tensor_mul(t1, t1, s)
        nc.vector.tensor_scalar(
            out=t1, in0=t1, scalar1=-0.5, scalar2=1.5,
            op0=ALU.mult, op1=ALU.add,
        )
        nc.vector.tensor_mul(y, y, t1)
    nc.vector.tensor_copy(out_rn, y)


def _emit(tc, p_stats, pt, p_blk, eye_in, lse_out):
    nc = tc.nc

    persist = tc.alloc_tile_pool(name="persist", bufs=1)
    pin = tc.alloc_tile_pool(name="pin", bufs=2)
    work = tc.alloc_tile_pool(name="work", bufs=2)
    dram = tc.alloc_tile_pool(name="dram", bufs=1, space="DRAM")
    epool = tc.alloc_tile_pool(name="epool", bufs=2)

    # Persistent tensors
    qt0 = persist.tile([128, B], BF16, tag="qt0", name="qt0")
    qt1 = persist.tile([128, B], BF16, tag="qt1", name="qt1")
    bt0 = persist.tile([128, BLK], BF16, tag="bt0", name="bt0")
    bt1 = persist.tile([128, BLK], BF16, tag="bt1", name="bt1")
    q_b = persist.tile([128, M_TILES, D], BF16, tag="q_b", name="q_b")
    rn_f = persist.tile([128, 64], F32, tag="rn_f", name="rn_f")
    rn_b = persist.tile([128, M_TILES], F32, tag="rn_b", name="rn_b")
    selfdot = persist.tile([128, M_TILES], F32, tag="selfdot", name="selfdot")
    sums = persist.tile([128, N_GROUPS * M_TILES], F32, tag="sums", name="sums")
    rowsum = persist.tile([128, M_TILES], F32, tag="rowsum", name="rowsum")
    exps = persist.tile([128, M_TILES], F32, tag="exps", name="exps")
    lse = persist.tile([128, M_TILES], F32, tag="lse", name="lse")
    dram_rn = dram.tile([B], F32, tag="dram_rn", name="dram_rn")

    # ---- This core's row block: norms, scale, self-dot, transpose ----
    pb = p_blk.rearrange("(t p) d -> t p d", p=128)    # [8, 128, 256]
    blk = persist.tile([128, M_TILES, D], F32, tag="blk", name="blk")
    eye = persist.tile([128, 128], BF16, tag="eye", name="eye")
    nc.gpsimd.dma_start(out=eye, in_=eye_in)
    for t in range(M_TILES):
        nc.gpsimd.dma_start(out=blk[:, t, :], in_=pb[t])
    sq_b = work.tile([128, M_TILES, D], BF16, name="sq_b", tag="sq_b", bufs=1)
    nc.vector.tensor_mul(sq_b, blk, blk)
    stats_b = work.tile([128, M_TILES], F32, name="stats_b", tag="stats_b", bufs=1)
    nc.vector.tensor_reduce(stats_b, sq_b, axis=mybir.AxisListType.X, op=ALU.add)
    _newton_rsqrt(nc, work, rn_b, stats_b)
    for t in range(M_TILES):
        nc.vector.tensor_scalar_mul(q_b[:, t, :], blk[:, t, :], rn_b[:, t : t + 1])
    sq_b2 = work.tile([128, M_TILES, D], BF16, name="sq_b2", tag="sq_b", bufs=1)
    nc.vector.tensor_mul(sq_b2, q_b, q_b)
    nc.vector.tensor_reduce(selfdot, sq_b2, axis=mybir.AxisListType.X, op=ALU.add)
    # Transpose the block on the (otherwise idle) tensor engine; copy the
    # PSUM results to SBUF on the scalar engine.  This keeps the slow DMA
    # xbar out of the picture and frees the main loop from DMA-queue deps.
    tp_psum = tc.alloc_tile_pool(name="tp_psum", bufs=4, space="PSUM")
    for t in range(M_TILES):
        for half, btk in ((0, bt0), (1, bt1)):
            tp = tp_psum.tile([128, 128], BF16, name="tp")
            nc.tensor.transpose(tp, q_b[:, t, half * 128 : half * 128 + 128], eye)
            nc.scalar.copy(out=btk[:, t * 128 : (t + 1) * 128], in_=tp)
    tp_psum.release()
    psum_pool = tc.alloc_tile_pool(name="psum", bufs=2, space="PSUM")

    # ---- Full-matrix norms + scaled transposed operand, one group at a
    # time (group g covers columns [2048g, 2048(g+1)) = rows with the
    # same indices; the u=16 interleave keeps j-order identity) ----
    # stats load: row j = 2048t + 16p + u  ->  tile t, partition p, slot u
    ps4 = p_stats.rearrange("(t p u) d -> t p (u d)", p=128, u=U)  # [4,128,4096]
    # rn store: dram_rn[2048t + 16p + u] <- rn_small[p, 16t + u]
    rn_store = dram_rn.rearrange("(t p u) -> t p u", p=128, u=U)   # [4,128,16]

    def normalize_group(g):
        pst = pin.tile([128, U * D], F32, name="pst", tag="pst", bufs=2)
        nc.sync.dma_start(out=pst, in_=ps4[g])
        sq = work.tile([128, U * D], BF16, name="sq", tag="sq", bufs=2)
        nc.vector.tensor_mul(sq, pst, pst)
        nc.vector.tensor_reduce(
            rn_f[:, g * U : (g + 1) * U],
            sq.rearrange("p (u d) -> p u d", u=U),
            axis=mybir.AxisListType.X,
            op=ALU.add,
        )
        _newton_rsqrt(
            nc, work, rn_f[:, g * U : (g + 1) * U], rn_f[:, g * U : (g + 1) * U]
        )
        nc.sync.dma_start(
            out=rn_store[g],
            in_=rn_f[:, g * U : (g + 1) * U].rearrange("p (t u) -> p t u", u=U),
        )
        rnb = work.tile([128, GROUP], F32, name="rnb", tag="rnb", bufs=2)
        nc.sync.dma_start(
            out=rnb,
            in_=dram_rn[g * GROUP : (g + 1) * GROUP].partition_broadcast(128),
        )
        for k, qtk in enumerate((qt0, qt1)):
            ptc = pin.tile([128, GROUP], F32, name="ptc", tag="ptc", bufs=4)
            nc.gpsimd.dma_start(
                out=ptc,
                in_=pt[k * 128 : (k + 1) * 128, g * GROUP : (g + 1) * GROUP],
            )
            nc.vector.tensor_mul(
                qtk[:, g * GROUP : (g + 1) * GROUP], ptc, rnb
            )

    normalize_group(0)

    # ---- Main loop: S-block matmuls + fused exp/row-sum ----
    for g in range(N_GROUPS):
        if g + 1 < N_GROUPS:
            normalize_group(g + 1)
        for m in range(M_TILES):
            ps = psum_pool.tile([128, GROUP], F32, name="ps")
            for n4 in range(N_PER_GROUP):
                col = g * GROUP + n4 * N_COLS
                for k, (btk, qtk) in enumerate(((bt0, qt0), (bt1, qt1))):
                    nc.tensor.matmul(
                        ps[:, n4 * N_COLS : (n4 + 1) * N_COLS],
                        btk[:, m * 128 : (m + 1) * 128],
                        qtk[:, col : col + N_COLS],
                        start=(k == 0),
                        stop=(k == 1),
                    )
            esc = epool.tile([128, GROUP], BF16, name="esc")
            nc.scalar.activation(
                out=esc,
                in_=ps,
                func=AF.Exp,
                scale=2.0,
                accum_out=sums[:, g * M_TILES + m : g * M_TILES + m + 1],
            )

    # ---- Epilogue: rowsum over groups, drop diagonal, log, write out ----
    sums3 = sums.rearrange("p (g m) -> p m g", g=N_GROUPS)
    nc.vector.tensor_reduce(rowsum, sums3, axis=mybir.AxisListType.X, op=ALU.add)
    nc.scalar.activation(out=exps, in_=selfdot, func=AF.Exp, scale=2.0)
    nc.vector.tensor_tensor(lse, rowsum, exps, op=ALU.subtract)
    nc.scalar.activation(out=lse, in_=lse, func=AF.Ln)
    nc.sync.dma_start(out=lse_out, in_=lse)

    for p in (epool, psum_pool, dram, work, pin, persist):
        p.release()


_BUILT = None


def _build():
    global _BUILT
    if _BUILT is None:
        nc = bacc.Bacc("TRN2", target_bir_lowering=False, debug=False,
                       num_devices=N_CORES)
        p_stats = nc.dram_tensor("p_stats", [B, D], F32, kind="ExternalInput").ap()
        pt = nc.dram_tensor("pt", [D, B], F32, kind="ExternalInput").ap()
        eye = nc.dram_tensor("eye", [128, 128], BF16, kind="ExternalInput").ap()
        p_blk = nc.dram_tensor("p_blk", [BLK, D], F32, kind="ExternalInput").ap()
        lse_out = nc.dram_tensor("lse_out", [128, M_TILES], F32,
                                 kind="ExternalOutput").ap()
        with tile.TileContext(nc) as tc:
            _emit(tc, p_stats, pt, p_blk, eye, lse_out)
        nc.finalize()
        _BUILT = nc
    return _BUILT


def run_on_hw(P, **spmd_kwargs):
    import jax.numpy as jnp

    nc = _build()
    pt_f32 = np.ascontiguousarray(P.T)
    eye = np.asarray(jnp.eye(128, dtype=jnp.bfloat16))
    in_maps = [
        {
            "p_stats": P,
            "pt": pt_f32,
            "p_blk": np.ascontiguousarray(P[c * BLK : (c + 1) * BLK]),
            "eye": eye,
        }
        for c in range(N_CORES)
    ]
    return bass_utils.run_bass_kernel_spmd(
        nc, in_maps, core_ids=list(range(N_CORES)), **spmd_kwargs
    )


def kernel(embedding1, embedding2, projection1, projection2):
    import jax.numpy as jnp

    # embeddings are unused by the reference computation
    P = np.ascontiguousarray(
        np.concatenate([projection1, projection2], axis=0), dtype=np.float32
    )
    res = run_on_hw(P)
    # reassemble per-row lse: core c, tile column m, partition p ->
    # global row c*1024 + m*128 + p
    lse_rows = np.empty(B, np.float32)
    for c in range(N_CORES):
        arr = np.asarray(res.results[c]["lse_out"])  # [128, M_TILES]
        lse_rows[c * BLK : (c + 1) * BLK] = arr.T.reshape(-1)
    # Reference fp32 semantics: logp_ii = f32(-2e9 - lse_i) (== -2e9 for
    # any |lse| < 128), then loss = -mean(logp) with the platform's XLA
    # fp32 reduction -- reproduce it bit-for-bit.
    logp = (np.float32(-2.0e9) - lse_rows).astype(np.float32)
    loss = -jnp.mean(jnp.asarray(logp))
    return np.asarray(loss)



# revision 7
# speedup vs baseline: 1.4884x; 1.4884x over previous
"""Contrastive (SimCLR-style) loss on 8 Trainium2 NeuronCores.

Math (matches the reference exactly):
  P = concat(projection1, projection2)            # [8192, 256]
  sim = cos_sim(P_i, P_j); diag masked to -1e9; logits = sim / 0.5
  labels = arange(2B)  -> picks the masked diagonal, so
  loss = -mean_i( logp_ii ),  logp_ii = f32(-2e9 - lse_i),
  lse_i = log(sum_{j != i} exp(2*sim_ij))

Distribution (mirrors the data-parallel sharding hint): the 8192 rows are
sharded 1024/core.  Each core receives the "all-gathered" normalized
projection matrix Q^T as an fp8 DoubleRow-interleaved operand, plus its own
RAW row block (fp8 transposed for the matmul weights + bf16 row-major for
norms).  On chip, each core:
  - computes its block's row norms (DVE square/reduce + Newton rsqrt),
  - runs fp8 DoubleRow matmuls (K=256 per instruction) computing
    d_ij = p_i . q_j into PSUM,
  - applies exp((2/n_i) * d_ij) with the row norm folded into the
    activation's per-partition scale; row-sums come from the activation
    accumulator (ScalarE) and, for a tunable fraction of columns, from a
    Schraudolph bit-trick exp on the DVE (int16/bf16 domain) with a fused
    tensor_tensor_reduce,
  - subtracts the diagonal term and takes log via an inverse bit-trick.
Host applies the reference's fp32 arithmetic for the final mean.
"""

import sys

for _p in ("/opt/trn_rl_repo", "/root/.axon_site/_ro/trn_rl_repo"):
    if _p not in sys.path:
        sys.path.append(_p)

import numpy as np
import ml_dtypes

import concourse.bacc as bacc
import concourse.tile as tile
from concourse import mybir
from concourse import bass_utils

F32 = mybir.dt.float32
BF16 = mybir.dt.bfloat16
FP8 = mybir.dt.float8e4
I16 = mybir.dt.int16
I32 = mybir.dt.int32
AF = mybir.ActivationFunctionType
ALU = mybir.AluOpType
PERF = mybir.MatmulPerfMode

N_CORES = 8
B = 8192          # total rows (2 * batch)
D = 256           # projection dim
BLK = B // N_CORES        # 1024 rows per core
M_TILES = BLK // 128      # 8 row tiles per core
N_GROUPS = 4              # column groups of 2048
GROUP = B // N_GROUPS     # 2048

# Schraudolph exp/log constants (f32 bit domain)
A_EXP = 12102203.161561485      # 2^23 / ln 2
B_EXP = 1064866805.0            # 127*2^23 - mean-centering offset
A16 = A_EXP / 65536.0           # int16/bf16-domain variants
B16 = B_EXP / 65536.0
E2 = 7.38905609893065           # exp(2): diagonal term to subtract
RSQRT_MAGIC = 0x5F3759DF

# Per-(m,g) cell split of the 2048 columns into ScalarE/DVE chunks.
# 'A' chunks go through the exp activation (<=1536 cols, 3 PSUM banks),
# 'D' chunks (512 cols, 1 bank) go through the DVE bit-exp path.
P1 = (("A", 1536), ("D", 512))
P2 = (("A", 1024), ("D", 512), ("D", 512))
P3 = (("D", 512),) * 4


ACT_ONLY = True


def _cell_patterns():
    grid = {}
    for m in range(M_TILES):
        for g in range(N_GROUPS):
            if ACT_ONLY:
                grid[(m, g)] = (("A", 1024), ("A", 1024))
            elif m in (1, 4) and g in (0, 2):
                grid[(m, g)] = P3
            else:
                grid[(m, g)] = P1
    return grid


CELLS = _cell_patterns()


def _newton_rsqrt(nc, pool, out_rn, s, final_scale=1.0):
    """out_rn = final_scale/sqrt(s), entirely on VectorE (fp32)."""
    p, w = s.shape
    ibits = pool.tile([p, w], I32, name="ibits", tag="rsq_i", bufs=1)
    nc.vector.tensor_scalar(
        out=ibits, in0=s.bitcast(I32), scalar1=1, scalar2=None,
        op0=ALU.arith_shift_right,
    )
    nc.vector.tensor_scalar(
        out=ibits, in0=ibits, scalar1=-1, scalar2=RSQRT_MAGIC,
        op0=ALU.mult, op1=ALU.add,
    )
    y = ibits.bitcast(F32)
    t1 = pool.tile([p, w], F32, name="t1", tag="rsq_t1", bufs=1)
    for it in range(2):
        nc.vector.tensor_mul(t1, y, y)
        nc.vector.tensor_mul(t1, t1, s)
        nc.vector.tensor_scalar(
            out=t1, in0=t1, scalar1=-0.5, scalar2=1.5,
            op0=ALU.mult, op1=ALU.add,
        )
        if it == 0:
            nc.vector.tensor_mul(y, y, t1)
        else:
            nc.vector.tensor_mul(t1, y, t1)
    # t1 holds 1/sqrt(s)
    nc.vector.tensor_scalar_mul(out_rn, t1, float(final_scale))


def _emit(tc, qt8, wt8, pb, lse_out):
    nc = tc.nc

    persist = tc.alloc_tile_pool(name="persist", bufs=1)
    work = tc.alloc_tile_pool(name="work", bufs=2)
    epool = tc.alloc_tile_pool(name="epool", bufs=2)

    # ---- SBUF persistent tensors ----
    qt_s = persist.tile([128, 2, B], FP8, tag="qt_s", name="qt_s")
    wt_s = persist.tile([128, 2, BLK], FP8, tag="wt_s", name="wt_s")
    pb_s = persist.tile([128, M_TILES, D], BF16, tag="pb_s", name="pb_s")
    rn2 = persist.tile([128, M_TILES], F32, tag="rn2", name="rn2")
    cexp = persist.tile([128, M_TILES], F32, tag="cexp", name="cexp")
    sums = persist.tile([128, M_TILES, 8], F32, tag="sums", name="sums")
    lse = persist.tile([128, M_TILES], F32, tag="lse", name="lse")

    # ---- DMA loads (sync queue; order sets priority) ----
    nc.sync.dma_start(out=wt_s, in_=wt8)
    nc.sync.dma_start(out=pb_s, in_=pb)
    for g in range(N_GROUPS):
        nc.sync.dma_start(
            out=qt_s[:, :, g * GROUP : (g + 1) * GROUP],
            in_=qt8[:, :, g * GROUP : (g + 1) * GROUP],
        )

    nc.vector.memset(sums, 0.0)

    # ---- Prologue: own-block row norms -> activation scales ----
    sq = work.tile([128, M_TILES, D], BF16, name="sq", tag="sq", bufs=1)
    nc.vector.tensor_mul(sq, pb_s, pb_s)
    nsq = work.tile([128, M_TILES], F32, name="nsq", tag="nsq", bufs=1)
    nc.vector.tensor_reduce(nsq, sq, axis=mybir.AxisListType.X, op=ALU.add)
    # rn2 = 2/n_i (activation scale), cexp = (2*A16)/n_i (bit-exp scale)
    _newton_rsqrt(nc, work, rn2, nsq, final_scale=2.0)
    nc.vector.tensor_scalar_mul(cexp, rn2, float(A16))

    # ---- PSUM pools: ScalarE slots (2x3 banks) + DVE slots (2x1 bank) ----
    ps_act = tc.alloc_tile_pool(name="ps_act", bufs=2, space="PSUM")
    ps_dve = tc.alloc_tile_pool(name="ps_dve", bufs=2, space="PSUM")

    zeros16 = persist.tile([128, 512], I16, tag="zeros16", name="zeros16")
    nc.vector.memset(zeros16, 0)

    # ---- Main loop: m-outer (weights stay stationary), g-inner ----
    for m in range(M_TILES):
        wslice = wt_s[:, :, m * 128 : (m + 1) * 128]
        pend = []          # pending DVE bit-exp chunks of this m-tile
        slot = [0]         # running partial-sum slot index (max 8 per m)

        def flush_pair(force=False):
            while len(pend) >= 2 or (force and pend):
                e0 = pend.pop(0)
                e1 = pend.pop(0) if pend else zeros16
                outp = epool.tile([128, 512], BF16, name="tts_o", tag="tts_o",
                                  bufs=2)
                nc.vector.tensor_tensor_reduce(
                    out=outp,
                    in0=e0.bitcast(BF16),
                    in1=e1.bitcast(BF16),
                    scale=1.0,
                    scalar=0.0,
                    op0=ALU.add,
                    op1=ALU.add,
                    accum_out=sums[:, m, slot[0] : slot[0] + 1],
                )
                slot[0] += 1

        for g in range(N_GROUPS):
            col0 = g * GROUP
            off = 0
            for kind, ncols in CELLS[(m, g)]:
                if kind == "A":
                    ps = ps_act.tile([128, 1536], F32, name="ps_a")
                    for n4 in range(ncols // 512):
                        c = col0 + off + n4 * 512
                        nc.tensor.matmul(
                            ps[:, n4 * 512 : (n4 + 1) * 512],
                            wslice,
                            qt_s[:, :, c : c + 512],
                            start=True, stop=True,
                            perf_mode=PERF.DoubleRow,
                        )
                    esc = epool.tile([128, 1536], BF16, name="esc", tag="esc",
                                     bufs=2)
                    nc.scalar.activation(
                        out=esc[:, :ncols],
                        in_=ps[:, :ncols],
                        func=AF.Exp,
                        scale=rn2[:, m : m + 1],
                        accum_out=sums[:, m, slot[0] : slot[0] + 1],
                    )
                    slot[0] += 1
                else:
                    ps = ps_dve.tile([128, 512], F32, name="ps_d")
                    c = col0 + off
                    nc.tensor.matmul(
                        ps, wslice, qt_s[:, :, c : c + 512],
                        start=True, stop=True,
                        perf_mode=PERF.DoubleRow,
                    )
                    eb = epool.tile([128, 512], I16, name="eb", tag="eb",
                                    bufs=4)
                    nc.vector.tensor_scalar(
                        out=eb, in0=ps,
                        scalar1=cexp[:, m : m + 1], scalar2=float(B16),
                        op0=ALU.mult, op1=ALU.add,
                    )
                    pend.append(eb)
                    flush_pair()
                off += ncols
        flush_pair(force=True)

    # ---- Epilogue ----
    rowsum = persist.tile([128, M_TILES], F32, tag="rowsum", name="rowsum")
    nc.vector.tensor_reduce(rowsum, sums, axis=mybir.AxisListType.X, op=ALU.add)
    nc.vector.tensor_scalar_add(rowsum, rowsum, -float(E2))
    # bit-trick ln: lse = (bits(x) - B_EXP)/A_EXP
    nc.vector.tensor_scalar(
        out=lse, in0=rowsum.bitcast(I32),
        scalar1=1.0 / A_EXP, scalar2=-B_EXP / A_EXP,
        op0=ALU.mult, op1=ALU.add,
    )
    nc.sync.dma_start(out=lse_out, in_=lse)

    for p in (epool, ps_dve, ps_act, work, persist):
        p.release()


_BUILT = None


def _build():
    global _BUILT
    if _BUILT is None:
        nc = bacc.Bacc("TRN2", target_bir_lowering=False, debug=False,
                       num_devices=N_CORES)
        qt8 = nc.dram_tensor("qt8", [128, 2, B], FP8, kind="ExternalInput").ap()
        wt8 = nc.dram_tensor("wt8", [128, 2, BLK], FP8,
                             kind="ExternalInput").ap()
        pb = nc.dram_tensor("pb", [128, M_TILES, D], BF16,
                            kind="ExternalInput").ap()
        lse_out = nc.dram_tensor("lse_out", [128, M_TILES], F32,
                                 kind="ExternalOutput").ap()
        with tile.TileContext(nc) as tc:
            _emit(tc, qt8, wt8, pb, lse_out)
        nc.finalize()
        _BUILT = nc
    return _BUILT


def _host_prep(P):
    """Host-side staging: normalized fp8 Q^T (DoubleRow-interleaved), raw
    fp8 block weights, bf16 row-major blocks for on-chip norms."""
    n = np.linalg.norm(P, axis=1, keepdims=True)
    Q = P / n
    # qt8[ki, ko, j] = Q[j, 128*ko + ki]
    qt8 = np.clip(Q.T, -240, 240).reshape(2, 128, B).transpose(1, 0, 2)
    qt8 = np.ascontiguousarray(qt8).astype(ml_dtypes.float8_e4m3)
    wt8s, pbs = [], []
    for c in range(N_CORES):
        Pb = P[c * BLK : (c + 1) * BLK]
        wt8 = np.clip(Pb.T, -240, 240).reshape(2, 128, BLK).transpose(1, 0, 2)
        wt8s.append(np.ascontiguousarray(wt8).astype(ml_dtypes.float8_e4m3))
        pb = Pb.reshape(M_TILES, 128, D).transpose(1, 0, 2)
        pbs.append(np.ascontiguousarray(pb).astype(ml_dtypes.bfloat16))
    return qt8, wt8s, pbs


def run_on_hw(P, **spmd_kwargs):
    nc = _build()
    qt8, wt8s, pbs = _host_prep(P)
    in_maps = [
        {"qt8": qt8, "wt8": wt8s[c], "pb": pbs[c]} for c in range(N_CORES)
    ]
    return bass_utils.run_bass_kernel_spmd(
        nc, in_maps, core_ids=list(range(N_CORES)), **spmd_kwargs
    )


DEBUG_LSE = None


def kernel(embedding1, embedding2, projection1, projection2):
    import jax.numpy as jnp

    global DEBUG_LSE
    # embeddings are unused by the reference computation
    P = np.ascontiguousarray(
        np.concatenate([projection1, projection2], axis=0), dtype=np.float32
    )
    res = run_on_hw(P)
    # reassemble per-row lse: core c, tile column t, partition p ->
    # global row c*1024 + t*128 + p
    lse_rows = np.empty(B, np.float32)
    for c in range(N_CORES):
        arr = np.asarray(res.results[c]["lse_out"])  # [128, M_TILES]
        lse_rows[c * BLK : (c + 1) * BLK] = arr.T.reshape(-1)
    DEBUG_LSE = lse_rows
    # Reference fp32 semantics: logp_ii = f32(-2e9 - lse_i) (== -2e9 for
    # any |lse| < 128), then loss = -mean(logp) with the platform's XLA
    # fp32 reduction -- reproduce it bit-for-bit.
    logp = (np.float32(-2.0e9) - lse_rows).astype(np.float32)
    loss = -jnp.mean(jnp.asarray(logp))
    return np.asarray(loss)


# revision 12
# speedup vs baseline: 1.9311x; 1.2974x over previous
"""Contrastive (SimCLR-style) loss on 8 Trainium2 NeuronCores.

Math (matches the reference exactly):
  P = concat(projection1, projection2)            # [8192, 256]
  sim = cos_sim(P_i, P_j); diag masked to -1e9; logits = sim / 0.5
  labels = arange(2B)  -> picks the masked diagonal, so
  loss = -mean_i( logp_ii ),  logp_ii = f32(-2e9 - lse_i),
  lse_i = log(sum_{j != i} exp(2*sim_ij))

Distribution (mirrors the data-parallel sharding hint): the 8192 rows are
sharded 1024/core.  Each core receives the "all-gathered" normalized
projection matrix Q^T as an fp8 DoubleRow-interleaved operand, plus its own
RAW row block (fp8 transposed for the matmul weights + bf16 row-major for
norms).  On chip, each core:
  - computes its block's row norms (DVE square/reduce + Newton rsqrt),
  - runs fp8 DoubleRow matmuls (K=256 per instruction) computing
    d_ij = p_i . q_j into PSUM,
  - applies exp((2/n_i) * d_ij) with the row norm folded into the
    activation's per-partition scale; row-sums come from the activation
    accumulator (ScalarE) and, for a tunable fraction of columns, from a
    Schraudolph bit-trick exp on the DVE (int16/bf16 domain) with a fused
    tensor_tensor_reduce,
  - subtracts the diagonal term and takes log via an inverse bit-trick.
Host applies the reference's fp32 arithmetic for the final mean.
"""

import sys

for _p in ("/opt/trn_rl_repo", "/root/.axon_site/_ro/trn_rl_repo"):
    if _p not in sys.path:
        sys.path.append(_p)

import numpy as np
import ml_dtypes

import concourse.bacc as bacc
import concourse.tile as tile
from concourse import mybir
from concourse import bass_utils

F32 = mybir.dt.float32
BF16 = mybir.dt.bfloat16
FP8 = mybir.dt.float8e4
I16 = mybir.dt.int16
I32 = mybir.dt.int32
AF = mybir.ActivationFunctionType
ALU = mybir.AluOpType
PERF = mybir.MatmulPerfMode

N_CORES = 8
B = 8192          # total rows (2 * batch)
D = 256           # projection dim
BLK = B // N_CORES        # 1024 rows per core
M_TILES = BLK // 128      # 8 row tiles per core
N_GROUPS = 4              # column groups of 2048
GROUP = B // N_GROUPS     # 2048

# Schraudolph exp/log constants (f32 bit domain)
A_EXP = 12102203.161561485      # 2^23 / ln 2
B_EXP = 1064866805.0            # 127*2^23 - mean-centering offset
A16 = A_EXP / 65536.0           # int16/bf16-domain variants
B16 = B_EXP / 65536.0
E2 = 7.38905609893065           # exp(2): diagonal term to subtract
RSQRT_MAGIC = 0x5F3759DF

# Per-(m,g) cell split of the 2048 columns into ScalarE/DVE chunks.
# 'A' chunks go through the exp activation (<=1536 cols, 3 PSUM banks),
# 'D' chunks (512 cols, 1 bank) go through the DVE bit-exp path.
P1 = (("A", 1536), ("D", 512))
P2 = (("A", 1024), ("D", 512), ("D", 512))
P3 = (("D", 512),) * 4


ACT_ONLY = False


def _cell_patterns():
    grid = {}
    for m in range(M_TILES):
        for g in range(N_GROUPS):
            if ACT_ONLY:
                grid[(m, g)] = (("A", 1024), ("A", 1024))
            elif m in (1, 4) and g in (0, 2):
                grid[(m, g)] = P3
            else:
                grid[(m, g)] = P1
    return grid


CELLS = _cell_patterns()


def _newton_rsqrt(nc, pool, out_rn, s, final_scale=1.0):
    """out_rn = final_scale/sqrt(s), entirely on VectorE (fp32)."""
    p, w = s.shape
    ibits = pool.tile([p, w], I32, name="ibits", tag="rsq_i", bufs=1)
    nc.vector.tensor_scalar(
        out=ibits, in0=s.bitcast(I32), scalar1=1, scalar2=None,
        op0=ALU.arith_shift_right,
    )
    nc.vector.tensor_scalar(
        out=ibits, in0=ibits, scalar1=-1, scalar2=RSQRT_MAGIC,
        op0=ALU.mult, op1=ALU.add,
    )
    y = ibits.bitcast(F32)
    t1 = pool.tile([p, w], F32, name="t1", tag="rsq_t1", bufs=1)
    for it in range(2):
        nc.vector.tensor_mul(t1, y, y)
        nc.vector.tensor_mul(t1, t1, s)
        nc.vector.tensor_scalar(
            out=t1, in0=t1, scalar1=-0.5, scalar2=1.5,
            op0=ALU.mult, op1=ALU.add,
        )
        if it == 0:
            nc.vector.tensor_mul(y, y, t1)
        else:
            nc.vector.tensor_mul(t1, y, t1)
    # t1 holds 1/sqrt(s)
    nc.vector.tensor_scalar_mul(out_rn, t1, float(final_scale))


def _emit(tc, qt8, wt8, pb, lse_out):
    nc = tc.nc

    persist = tc.alloc_tile_pool(name="persist", bufs=1)
    work = tc.alloc_tile_pool(name="work", bufs=2)
    epool = tc.alloc_tile_pool(name="epool", bufs=2)

    # ---- SBUF persistent tensors ----
    qt_s = persist.tile([128, 2, B], FP8, tag="qt_s", name="qt_s")
    wt_s = persist.tile([128, 2, BLK], FP8, tag="wt_s", name="wt_s")
    pb_s = persist.tile([128, M_TILES, D], BF16, tag="pb_s", name="pb_s")
    rn2 = persist.tile([128, M_TILES], F32, tag="rn2", name="rn2")
    cexp = persist.tile([128, M_TILES], F32, tag="cexp", name="cexp")
    sums = persist.tile([128, M_TILES, 16], F32, tag="sums", name="sums")
    lse = persist.tile([128, M_TILES], F32, tag="lse", name="lse")

    # ---- DMA loads (sync queue; order sets priority) ----
    nc.sync.dma_start(out=wt_s, in_=wt8)
    nc.sync.dma_start(out=pb_s, in_=pb)
    for g in range(N_GROUPS):
        nc.sync.dma_start(
            out=qt_s[:, :, g * GROUP : (g + 1) * GROUP],
            in_=qt8[:, :, g * GROUP : (g + 1) * GROUP],
        )

    nc.vector.memset(sums, 0.0)

    # ---- Prologue: own-block row norms -> activation scales ----
    sq = work.tile([128, M_TILES, D], BF16, name="sq", tag="sq", bufs=1)
    nc.vector.tensor_mul(sq, pb_s, pb_s)
    nsq = work.tile([128, M_TILES], F32, name="nsq", tag="nsq", bufs=1)
    nc.vector.tensor_reduce(nsq, sq, axis=mybir.AxisListType.X, op=ALU.add)
    # rn2 = 2/n_i (activation scale), cexp = (2*A_EXP)/n_i (bit-exp scale)
    _newton_rsqrt(nc, work, rn2, nsq, final_scale=2.0)
    nc.vector.tensor_scalar_mul(cexp, rn2, float(A_EXP))

    # ---- PSUM pools: ScalarE slots (2x3 banks) + DVE slots (2x1 bank) ----
    ps_act = tc.alloc_tile_pool(name="ps_act", bufs=2, space="PSUM")
    ps_dve = tc.alloc_tile_pool(name="ps_dve", bufs=2, space="PSUM")

    zeros16 = persist.tile([128, 512], I16, tag="zeros16", name="zeros16")
    nc.vector.memset(zeros16, 0)

    # ---- Main loop: m-outer (weights stay stationary), g-inner ----
    for m in range(M_TILES):
        wslice = wt_s[:, :, m * 128 : (m + 1) * 128]
        slot = [0]         # running partial-sum slot index (max 16 per m)

        for g in range(N_GROUPS):
            col0 = g * GROUP
            off = 0
            for kind, ncols in CELLS[(m, g)]:
                if kind == "A":
                    ps = ps_act.tile([128, 1536], F32, name="ps_a")
                    for n4 in range(ncols // 512):
                        c = col0 + off + n4 * 512
                        nc.tensor.matmul(
                            ps[:, n4 * 512 : (n4 + 1) * 512],
                            wslice,
                            qt_s[:, :, c : c + 512],
                            start=True, stop=True,
                            perf_mode=PERF.DoubleRow,
                        )
                    esc = epool.tile([128, 1536], BF16, name="esc", tag="esc",
                                     bufs=2)
                    nc.scalar.activation(
                        out=esc[:, :ncols],
                        in_=ps[:, :ncols],
                        func=AF.Exp,
                        scale=rn2[:, m : m + 1],
                        accum_out=sums[:, m, slot[0] : slot[0] + 1],
                    )
                    slot[0] += 1
                else:
                    ps = ps_dve.tile([128, 512], F32, name="ps_d")
                    c = col0 + off
                    nc.tensor.matmul(
                        ps, wslice, qt_s[:, :, c : c + 512],
                        start=True, stop=True,
                        perf_mode=PERF.DoubleRow,
                    )
                    eb = epool.tile([128, 512], I32, name="eb", tag="eb",
                                    bufs=4)
                    nc.vector.tensor_scalar(
                        out=eb, in0=ps,
                        scalar1=cexp[:, m : m + 1], scalar2=float(B_EXP),
                        op0=ALU.mult, op1=ALU.add,
                    )
                    nc.vector.tensor_reduce(
                        sums[:, m, slot[0] : slot[0] + 1],
                        eb.bitcast(F32),
                        axis=mybir.AxisListType.X,
                        op=ALU.add,
                    )
                    slot[0] += 1
                off += ncols

    # ---- Epilogue ----
    rowsum = persist.tile([128, M_TILES], F32, tag="rowsum", name="rowsum")
    nc.vector.tensor_reduce(rowsum, sums, axis=mybir.AxisListType.X, op=ALU.add)
    nc.vector.tensor_scalar_add(rowsum, rowsum, -float(E2))
    # bit-trick ln: lse = (bits(x) - B_EXP)/A_EXP
    nc.vector.tensor_scalar(
        out=lse, in0=rowsum.bitcast(I32),
        scalar1=1.0 / A_EXP, scalar2=-B_EXP / A_EXP,
        op0=ALU.mult, op1=ALU.add,
    )
    nc.sync.dma_start(out=lse_out, in_=lse)

    for p in (epool, ps_dve, ps_act, work, persist):
        p.release()


_BUILT = None


def _build():
    global _BUILT
    if _BUILT is None:
        nc = bacc.Bacc("TRN2", target_bir_lowering=False, debug=False,
                       num_devices=N_CORES)
        qt8 = nc.dram_tensor("qt8", [128, 2, B], FP8, kind="ExternalInput").ap()
        wt8 = nc.dram_tensor("wt8", [128, 2, BLK], FP8,
                             kind="ExternalInput").ap()
        pb = nc.dram_tensor("pb", [128, M_TILES, D], BF16,
                            kind="ExternalInput").ap()
        lse_out = nc.dram_tensor("lse_out", [128, M_TILES], F32,
                                 kind="ExternalOutput").ap()
        with tile.TileContext(nc) as tc:
            _emit(tc, qt8, wt8, pb, lse_out)
        nc.finalize()
        _BUILT = nc
    return _BUILT


def _host_prep(P):
    """Host-side staging: normalized fp8 Q^T (DoubleRow-interleaved), raw
    fp8 block weights, bf16 row-major blocks for on-chip norms."""
    n = np.linalg.norm(P, axis=1, keepdims=True)
    Q = P / n
    # qt8[ki, ko, j] = Q[j, 128*ko + ki]
    qt8 = np.clip(Q.T, -240, 240).reshape(2, 128, B).transpose(1, 0, 2)
    qt8 = np.ascontiguousarray(qt8).astype(ml_dtypes.float8_e4m3)
    wt8s, pbs = [], []
    for c in range(N_CORES):
        Pb = P[c * BLK : (c + 1) * BLK]
        wt8 = np.clip(Pb.T, -240, 240).reshape(2, 128, BLK).transpose(1, 0, 2)
        wt8s.append(np.ascontiguousarray(wt8).astype(ml_dtypes.float8_e4m3))
        pb = Pb.reshape(M_TILES, 128, D).transpose(1, 0, 2)
        pbs.append(np.ascontiguousarray(pb).astype(ml_dtypes.bfloat16))
    return qt8, wt8s, pbs


def run_on_hw(P, **spmd_kwargs):
    nc = _build()
    qt8, wt8s, pbs = _host_prep(P)
    in_maps = [
        {"qt8": qt8, "wt8": wt8s[c], "pb": pbs[c]} for c in range(N_CORES)
    ]
    return bass_utils.run_bass_kernel_spmd(
        nc, in_maps, core_ids=list(range(N_CORES)), **spmd_kwargs
    )


DEBUG_LSE = None


def kernel(embedding1, embedding2, projection1, projection2):
    import jax.numpy as jnp

    global DEBUG_LSE
    # embeddings are unused by the reference computation
    P = np.ascontiguousarray(
        np.concatenate([projection1, projection2], axis=0), dtype=np.float32
    )
    res = run_on_hw(P)
    # reassemble per-row lse: core c, tile column t, partition p ->
    # global row c*1024 + t*128 + p
    lse_rows = np.empty(B, np.float32)
    for c in range(N_CORES):
        arr = np.asarray(res.results[c]["lse_out"])  # [128, M_TILES]
        lse_rows[c * BLK : (c + 1) * BLK] = arr.T.reshape(-1)
    DEBUG_LSE = lse_rows
    # Reference fp32 semantics: logp_ii = f32(-2e9 - lse_i) (== -2e9 for
    # any |lse| < 128), then loss = -mean(logp) with the platform's XLA
    # fp32 reduction -- reproduce it bit-for-bit.
    logp = (np.float32(-2.0e9) - lse_rows).astype(np.float32)
    loss = -jnp.mean(jnp.asarray(logp))
    return np.asarray(loss)


# revision 15
# speedup vs baseline: 2.0604x; 1.0669x over previous
"""Contrastive (SimCLR-style) loss on 8 Trainium2 NeuronCores.

Math (matches the reference exactly):
  P = concat(projection1, projection2)            # [8192, 256]
  sim = cos_sim(P_i, P_j); diag masked to -1e9; logits = sim / 0.5
  labels = arange(2B)  -> picks the masked diagonal, so
  loss = -mean_i( logp_ii ),  logp_ii = f32(-2e9 - lse_i),
  lse_i = log(sum_{j != i} exp(2*sim_ij))

Distribution (mirrors the data-parallel sharding hint): the 8192 rows are
sharded 1024/core.  Each core receives the "all-gathered" normalized
projection matrix Q^T as an fp8 DoubleRow-interleaved operand, plus its own
RAW row block (fp8 transposed for the matmul weights + bf16 row-major for
norms).  On chip, each core:
  - computes its block's row norms (DVE square/reduce + Newton rsqrt),
  - runs fp8 DoubleRow matmuls (K=256 per instruction) computing
    d_ij = p_i . q_j into PSUM,
  - applies exp((2/n_i) * d_ij) with the row norm folded into the
    activation's per-partition scale; row-sums come from the activation
    accumulator (ScalarE) and, for a tunable fraction of columns, from a
    Schraudolph bit-trick exp on the DVE (int16/bf16 domain) with a fused
    tensor_tensor_reduce,
  - subtracts the diagonal term and takes log via an inverse bit-trick.
Host applies the reference's fp32 arithmetic for the final mean.
"""

import sys

for _p in ("/opt/trn_rl_repo", "/root/.axon_site/_ro/trn_rl_repo"):
    if _p not in sys.path:
        sys.path.append(_p)

import numpy as np
import ml_dtypes

import concourse.bacc as bacc
import concourse.tile as tile
from concourse import mybir
from concourse import bass_utils

F32 = mybir.dt.float32
BF16 = mybir.dt.bfloat16
FP8 = mybir.dt.float8e4
I16 = mybir.dt.int16
I32 = mybir.dt.int32
AF = mybir.ActivationFunctionType
ALU = mybir.AluOpType
PERF = mybir.MatmulPerfMode

N_CORES = 8
B = 8192          # total rows (2 * batch)
D = 256           # projection dim
BLK = B // N_CORES        # 1024 rows per core
M_TILES = BLK // 128      # 8 row tiles per core
N_GROUPS = 4              # column groups of 2048
GROUP = B // N_GROUPS     # 2048

# Schraudolph exp/log constants (f32 bit domain)
A_EXP = 12102203.161561485      # 2^23 / ln 2
B_EXP = 1064866805.0            # 127*2^23 - mean-centering offset
A16 = A_EXP / 65536.0           # int16/bf16-domain variants
B16 = B_EXP / 65536.0
E2 = 7.38905609893065           # exp(2): diagonal term to subtract
RSQRT_MAGIC = 0x5F3759DF

# Per-(m,g) cell split of the 2048 columns into ScalarE/DVE chunks.
# 'A' chunks go through the exp activation (<=1536 cols, 3 PSUM banks),
# 'D' chunks (512 cols, 1 bank) go through the DVE bit-exp path.
P1 = (("A", 1536), ("D", 512))
P2 = (("A", 1024), ("D", 512), ("D", 512))
P3 = (("D", 512),) * 4


ACT_ONLY = False


def _cell_patterns():
    grid = {}
    for m in range(M_TILES):
        for g in range(N_GROUPS):
            if ACT_ONLY:
                grid[(m, g)] = (("A", 1024), ("A", 1024))
            elif m == 1 and g in (0, 2):
                grid[(m, g)] = P3
            else:
                grid[(m, g)] = P1
    return grid


CELLS = _cell_patterns()


def _newton_rsqrt(nc, pool, out_rn, s, final_scale=1.0):
    """out_rn = final_scale/sqrt(s), entirely on VectorE (fp32)."""
    p, w = s.shape
    ibits = pool.tile([p, w], I32, name="ibits", tag="rsq_i", bufs=1)
    nc.vector.tensor_scalar(
        out=ibits, in0=s.bitcast(I32), scalar1=1, scalar2=None,
        op0=ALU.arith_shift_right,
    )
    nc.vector.tensor_scalar(
        out=ibits, in0=ibits, scalar1=-1, scalar2=RSQRT_MAGIC,
        op0=ALU.mult, op1=ALU.add,
    )
    y = ibits.bitcast(F32)
    t1 = pool.tile([p, w], F32, name="t1", tag="rsq_t1", bufs=1)
    for it in range(2):
        nc.vector.tensor_mul(t1, y, y)
        nc.vector.tensor_mul(t1, t1, s)
        nc.vector.tensor_scalar(
            out=t1, in0=t1, scalar1=-0.5, scalar2=1.5,
            op0=ALU.mult, op1=ALU.add,
        )
        if it == 0:
            nc.vector.tensor_mul(y, y, t1)
        else:
            nc.vector.tensor_mul(t1, y, t1)
    # t1 holds 1/sqrt(s)
    nc.vector.tensor_scalar_mul(out_rn, t1, float(final_scale))


def _emit(tc, qt8, wt8, pb, lse_out):
    nc = tc.nc

    persist = tc.alloc_tile_pool(name="persist", bufs=1)
    work = tc.alloc_tile_pool(name="work", bufs=2)
    epool = tc.alloc_tile_pool(name="epool", bufs=2)

    # ---- SBUF persistent tensors ----
    qt_s = persist.tile([128, 2, B], FP8, tag="qt_s", name="qt_s")
    wt_s = persist.tile([128, 2, BLK], FP8, tag="wt_s", name="wt_s")
    pb_s = persist.tile([128, M_TILES, D], BF16, tag="pb_s", name="pb_s")
    rn2 = persist.tile([128, M_TILES], F32, tag="rn2", name="rn2")
    cexp = persist.tile([128, M_TILES], F32, tag="cexp", name="cexp")
    sums = persist.tile([128, M_TILES, 16], F32, tag="sums", name="sums")
    lse = persist.tile([128, M_TILES], F32, tag="lse", name="lse")

    # ---- DMA loads (sync queue; order sets priority) ----
    nc.sync.dma_start(out=wt_s, in_=wt8)
    nc.sync.dma_start(out=pb_s, in_=pb)
    for g in range(N_GROUPS):
        nc.sync.dma_start(
            out=qt_s[:, :, g * GROUP : (g + 1) * GROUP],
            in_=qt8[:, :, g * GROUP : (g + 1) * GROUP],
        )

    nc.vector.memset(sums, 0.0)

    # ---- Prologue: own-block row norms -> activation scales ----
    sq = work.tile([128, M_TILES, D], BF16, name="sq", tag="sq", bufs=1)
    nc.vector.tensor_mul(sq, pb_s, pb_s)
    nsq = work.tile([128, M_TILES], F32, name="nsq", tag="nsq", bufs=1)
    nc.vector.tensor_reduce(nsq, sq, axis=mybir.AxisListType.X, op=ALU.add)
    # rn2 = 2/n_i (activation scale), cexp = (2*A_EXP)/n_i (bit-exp scale)
    _newton_rsqrt(nc, work, rn2, nsq, final_scale=2.0)
    nc.vector.tensor_scalar_mul(cexp, rn2, float(A_EXP))

    # ---- PSUM pools: ScalarE slots (2x3 banks) + DVE slots (2x1 bank) ----
    ps_act = tc.alloc_tile_pool(name="ps_act", bufs=2, space="PSUM")
    ps_dve = tc.alloc_tile_pool(name="ps_dve", bufs=2, space="PSUM")

    zeros16 = persist.tile([128, 512], I16, tag="zeros16", name="zeros16")
    nc.vector.memset(zeros16, 0)

    # ---- Main loop: m-outer (weights stay stationary), g-inner ----
    for m in range(M_TILES):
        wslice = wt_s[:, :, m * 128 : (m + 1) * 128]
        slot = [0]         # running partial-sum slot index (max 16 per m)
        pend = [None, 0]   # current paired i32 bit-exp buffer, fill count

        def flush_dve(force=False):
            eb2, nfill = pend
            if eb2 is None:
                return
            if nfill == 2:
                nc.vector.tensor_reduce(
                    sums[:, m, slot[0] : slot[0] + 2],
                    eb2.bitcast(F32),
                    axis=mybir.AxisListType.X,
                    op=ALU.add,
                )
                slot[0] += 2
                pend[0], pend[1] = None, 0
            elif force and nfill == 1:
                nc.vector.tensor_reduce(
                    sums[:, m, slot[0] : slot[0] + 1],
                    eb2[:, 0, :].bitcast(F32),
                    axis=mybir.AxisListType.X,
                    op=ALU.add,
                )
                slot[0] += 1
                pend[0], pend[1] = None, 0

        for g in range(N_GROUPS):
            col0 = g * GROUP
            off = 0
            for kind, ncols in CELLS[(m, g)]:
                if kind == "A":
                    ps = ps_act.tile([128, 1536], F32, name="ps_a")
                    for n4 in range(ncols // 512):
                        c = col0 + off + n4 * 512
                        nc.tensor.matmul(
                            ps[:, n4 * 512 : (n4 + 1) * 512],
                            wslice,
                            qt_s[:, :, c : c + 512],
                            start=True, stop=True,
                            perf_mode=PERF.DoubleRow,
                        )
                    esc = epool.tile([128, 1536], BF16, name="esc", tag="esc",
                                     bufs=2)
                    nc.scalar.activation(
                        out=esc[:, :ncols],
                        in_=ps[:, :ncols],
                        func=AF.Exp,
                        scale=rn2[:, m : m + 1],
                        accum_out=sums[:, m, slot[0] : slot[0] + 1],
                    )
                    slot[0] += 1
                else:
                    ps = ps_dve.tile([128, 512], F32, name="ps_d")
                    c = col0 + off
                    nc.tensor.matmul(
                        ps, wslice, qt_s[:, :, c : c + 512],
                        start=True, stop=True,
                        perf_mode=PERF.DoubleRow,
                    )
                    if pend[0] is None:
                        pend[0] = epool.tile([128, 2, 512], I32, name="eb",
                                             tag="eb", bufs=3)
                        pend[1] = 0
                    nc.vector.tensor_scalar(
                        out=pend[0][:, pend[1], :], in0=ps,
                        scalar1=cexp[:, m : m + 1], scalar2=float(B_EXP),
                        op0=ALU.mult, op1=ALU.add,
                    )
                    pend[1] += 1
                    flush_dve()
                off += ncols
        flush_dve(force=True)

    # ---- Epilogue ----
    rowsum = persist.tile([128, M_TILES], F32, tag="rowsum", name="rowsum")
    nc.vector.tensor_reduce(rowsum, sums, axis=mybir.AxisListType.X, op=ALU.add)
    nc.vector.tensor_scalar_add(rowsum, rowsum, -float(E2))
    # bit-trick ln: lse = (bits(x) - B_EXP)/A_EXP
    nc.vector.tensor_scalar(
        out=lse, in0=rowsum.bitcast(I32),
        scalar1=1.0 / A_EXP, scalar2=-B_EXP / A_EXP,
        op0=ALU.mult, op1=ALU.add,
    )
    nc.sync.dma_start(out=lse_out, in_=lse)

    for p in (epool, ps_dve, ps_act, work, persist):
        p.release()


_BUILT = None


def _build():
    global _BUILT
    if _BUILT is None:
        nc = bacc.Bacc("TRN2", target_bir_lowering=False, debug=False,
                       num_devices=N_CORES)
        qt8 = nc.dram_tensor("qt8", [128, 2, B], FP8, kind="ExternalInput").ap()
        wt8 = nc.dram_tensor("wt8", [128, 2, BLK], FP8,
                             kind="ExternalInput").ap()
        pb = nc.dram_tensor("pb", [128, M_TILES, D], BF16,
                            kind="ExternalInput").ap()
        lse_out = nc.dram_tensor("lse_out", [128, M_TILES], F32,
                                 kind="ExternalOutput").ap()
        with tile.TileContext(nc) as tc:
            _emit(tc, qt8, wt8, pb, lse_out)
        nc.finalize()
        _BUILT = nc
    return _BUILT


def _host_prep(P):
    """Host-side staging: normalized fp8 Q^T (DoubleRow-interleaved), raw
    fp8 block weights, bf16 row-major blocks for on-chip norms."""
    n = np.linalg.norm(P, axis=1, keepdims=True)
    Q = P / n
    # qt8[ki, ko, j] = Q[j, 128*ko + ki]
    qt8 = np.clip(Q.T, -240, 240).reshape(2, 128, B).transpose(1, 0, 2)
    qt8 = np.ascontiguousarray(qt8).astype(ml_dtypes.float8_e4m3)
    wt8s, pbs = [], []
    for c in range(N_CORES):
        Pb = P[c * BLK : (c + 1) * BLK]
        wt8 = np.clip(Pb.T, -240, 240).reshape(2, 128, BLK).transpose(1, 0, 2)
        wt8s.append(np.ascontiguousarray(wt8).astype(ml_dtypes.float8_e4m3))
        pb = Pb.reshape(M_TILES, 128, D).transpose(1, 0, 2)
        pbs.append(np.ascontiguousarray(pb).astype(ml_dtypes.bfloat16))
    return qt8, wt8s, pbs


def run_on_hw(P, **spmd_kwargs):
    nc = _build()
    qt8, wt8s, pbs = _host_prep(P)
    in_maps = [
        {"qt8": qt8, "wt8": wt8s[c], "pb": pbs[c]} for c in range(N_CORES)
    ]
    return bass_utils.run_bass_kernel_spmd(
        nc, in_maps, core_ids=list(range(N_CORES)), **spmd_kwargs
    )


DEBUG_LSE = None


def kernel(embedding1, embedding2, projection1, projection2):
    import jax.numpy as jnp

    global DEBUG_LSE
    # embeddings are unused by the reference computation
    P = np.ascontiguousarray(
        np.concatenate([projection1, projection2], axis=0), dtype=np.float32
    )
    res = run_on_hw(P)
    # reassemble per-row lse: core c, tile column t, partition p ->
    # global row c*1024 + t*128 + p
    lse_rows = np.empty(B, np.float32)
    for c in range(N_CORES):
        arr = np.asarray(res.results[c]["lse_out"])  # [128, M_TILES]
        lse_rows[c * BLK : (c + 1) * BLK] = arr.T.reshape(-1)
    DEBUG_LSE = lse_rows
    # Reference fp32 semantics: logp_ii = f32(-2e9 - lse_i) (== -2e9 for
    # any |lse| < 128), then loss = -mean(logp) with the platform's XLA
    # fp32 reduction -- reproduce it bit-for-bit.
    logp = (np.float32(-2.0e9) - lse_rows).astype(np.float32)
    loss = -jnp.mean(jnp.asarray(logp))
    return np.asarray(loss)
